# revision 16
# baseline (speedup 1.0000x reference)
"""Causal self-attention Trainium2 kernel.

Sharding: 8 cores = (4 batches) x (2 head-groups of 8 heads).
Each core: projections for its 512 channels, causal attention for its 8
heads over its batch, partial out-projection over its 512 channels.
Host: sums the two partials per batch and adds the output bias.

All matmul operands are bf16 (accumulation in fp32 PSUM); softmax,
normalization and the output partials stay fp32.

v2 structure (PE-density focused):
  - input DMAs split per 128-channel chunk and interleaved so the first
    projection matmul issues within a few us
  - attention is one flat software-pipelined stream: score-matmul unit
    u+1 issues before AV-matmul unit u, so the PE never waits on the
    ACT exp of the unit it just produced (keeps HAM clock at 2.4 GHz)
  - softmax normalization: reciprocal_approx_fast (DVE) + SBUF
    broadcast DMA + one DVE multiply; no PE broadcast matmul, no
    numerator copy

Layouts on core (b = fixed batch, channels o in [0,512) local):
  xT   [128f, 8fc, 2048t] bf16 - DMA transpose from DRAM, per fc chunk
  qT/kT [128o, 4oc, 2048t] bf16 - head h = oc*2+hh on partitions hh*64..+64
  vx   [128t, 16tj, 8h*65] bf16 - v natural + ones column per head
  scores^T tiles [128j, 512i] f32 psum -> exp on ACT (scale=1/8) -> bf16
  at accum psum [65, 512i] f32: rows 0..63 head out, row 64 denom
  out  psum [128t, 512c] f32 -> sbuf -> DRAM partial
"""

from contextlib import ExitStack

import ml_dtypes
import numpy as np

import concourse.bass as bass
import concourse.mybir as mybir
import concourse.tile as tile

P = 128
C = 1024  # d_model
CL = 512  # local channels (8 heads * 64)
D = 64  # head dim
NH = 8  # local heads
FC = C // P  # 8 f-chunks
OC = CL // P  # 4 o-chunks
F32 = mybir.dt.float32
BF16 = mybir.dt.bfloat16
AF = mybir.ActivationFunctionType
GROUP = 2  # score jt-tiles per exp call (2 psum banks, double buffered)
TAIL_DELAY = 7  # flushes between a block's last AV and its norm broadcast


def _emit(nc, tc, ctx, T):
    NT = T // P  # 128-token chunks
    T4 = T // 512  # 512-token chunks

    xb = nc.dram_tensor("xb", [T, C], BF16, kind="ExternalInput")
    wq_d = nc.dram_tensor("wq", [C, CL], BF16, kind="ExternalInput")
    wk_d = nc.dram_tensor("wk", [C, CL], BF16, kind="ExternalInput")
    wv_d = nc.dram_tensor("wv", [C, CL], BF16, kind="ExternalInput")
    wo_d = nc.dram_tensor("wo", [CL, C], BF16, kind="ExternalInput")
    bq_d = nc.dram_tensor("bq", [CL], F32, kind="ExternalInput")
    bk_d = nc.dram_tensor("bk", [CL], F32, kind="ExternalInput")
    bv_d = nc.dram_tensor("bv", [CL], BF16, kind="ExternalInput")
    stair_d = nc.dram_tensor("stair", [P, 1024], BF16, kind="ExternalInput")
    outp = nc.dram_tensor("outp", [T, C], F32, kind="ExternalOutput")

    const = ctx.enter_context(tc.tile_pool(name="const", bufs=1))
    ones1 = const.tile([1, P], BF16)
    nc.gpsimd.memset(ones1[:], 1.0)

    # q/k per head padded to 128 partitions (rows 64:128 zero) and the AV
    # stationary padded to 128 columns: the PE activity monitor only sees
    # full-row/column matmuls as "busy", and half-utilized matmuls leave
    # the clock gate at 1.2 GHz.  The zero padding costs no matmul time
    # (cost is column count) but keeps the array at 2.4 GHz.
    qkv = ctx.enter_context(tc.tile_pool(name="qkv", bufs=1))
    qTp = qkv.tile([P, NH, T], BF16)
    kTp = qkv.tile([P, NH, T], BF16)
    vxp = qkv.tile([P, NT, NH, P], BF16)
    nc.gpsimd.memset(qTp[D:P, :, :], 0.0)
    nc.gpsimd.memset(kTp[D:P, :, :], 0.0)
    nc.gpsimd.memset(vxp[:, :, :, 64:65], 1.0)
    nc.gpsimd.memset(vxp[:, :, :, 65:P], 0.0)

    # ---------------- projections ----------------
    with (
        tc.tile_pool(name="wpool", bufs=1) as wpool,
        tc.tile_pool(name="xT_pool", bufs=1) as xT_pool,
        tc.tile_pool(name="pj_ps", bufs=8, space="PSUM") as pj_ps,
    ):
        xT = xT_pool.tile([P, FC, T], BF16)
        wq_sb = wpool.tile([P, FC, CL], BF16)
        wk_sb = wpool.tile([P, FC, CL], BF16)
        wv_sb = wpool.tile([P, FC, CL], BF16)
        xbr = xb.rearrange("t (fc p) -> t fc p", p=P)
        wqr = wq_d.rearrange("(fc p) o -> fc p o", p=P)
        wkr = wk_d.rearrange("(fc p) o -> fc p o", p=P)
        wvr = wv_d.rearrange("(fc p) o -> fc p o", p=P)
        # interleave so chunk fc of x/wq/wk arrives before chunk fc+1,
        # alternating between the two hwdge rings (SP and ACT) so neither
        # ring's serial issue rate starves the PE.
        eng = [nc.sync, nc.scalar]
        for fc in range(FC):
            nc.sync.dma_start(xT[:, fc, :], xbr[:, fc, :], transpose=True)
            nc.scalar.dma_start(wq_sb[:, fc, :], wqr[fc])
            nc.scalar.dma_start(wk_sb[:, fc, :], wkr[fc])
        bq_sb = const.tile([P, OC], F32)
        nc.scalar.dma_start(bq_sb[:], bq_d.rearrange("(oc p) -> p oc", p=P))
        bk_sb = const.tile([P, OC], F32)
        nc.scalar.dma_start(bk_sb[:], bk_d.rearrange("(oc p) -> p oc", p=P))
        for fc in range(FC):
            eng[(fc + 1) % 2].dma_start(wv_sb[:, fc, :], wvr[fc])
        bv_sb = const.tile([1, CL], BF16)
        nc.scalar.dma_start(bv_sb[:], bv_d.rearrange("(a c) -> a c", a=1))
        stair_sb = const.tile([P, 1024], BF16)
        nc.scalar.dma_start(stair_sb[:], stair_d[:])

        for oc in range(OC):
            for w_sb, b_sb, dT in ((wq_sb, bq_sb, qTp), (wk_sb, bk_sb, kTp)):
                pss = [
                    pj_ps.tile([P, 512], F32, tag="pj", name=f"pj{oc}_{tt}")
                    for tt in range(T4)
                ]
                for fc in range(FC):
                    for tt in range(T4):
                        nc.tensor.matmul(
                            pss[tt][:],
                            w_sb[:, fc, oc * P : (oc + 1) * P],
                            xT[:, fc, tt * 512 : (tt + 1) * 512],
                            start=(fc == 0),
                            stop=(fc == FC - 1),
                        )
                for tt in range(T4):
                    for hh in range(2):
                        nc.vector.tensor_scalar_add(
                            dT[0:D, oc * 2 + hh, tt * 512 : (tt + 1) * 512],
                            pss[tt][hh * D : (hh + 1) * D, :],
                            b_sb[hh * D : (hh + 1) * D, oc : oc + 1],
                        )
        for s in range(NT):
            ps = pj_ps.tile([P, 512], F32, tag="pj", name=f"pjv{s}")
            for fc in range(FC):
                nc.tensor.matmul(
                    ps[:],
                    xT[:, fc, s * P : (s + 1) * P],
                    wv_sb[:, fc, :],
                    start=(fc == 0),
                    stop=False,
                )
            nc.tensor.matmul(
                ps[:],
                ones1[:],
                bv_sb[:],
                start=False,
                stop=True,
            )
            nc.vector.tensor_copy(
                vxp[:, s, :, 0:64],
                ps[:].rearrange("p (h d) -> p h d", d=D),
            )

    # ---------------- attention ----------------
    wo_pool = ctx.enter_context(tc.tile_pool(name="wo_pool", bufs=1))
    attT_pool = ctx.enter_context(tc.tile_pool(name="attT_pool", bufs=1))
    wo_sb = wo_pool.tile([P, OC, C], BF16)
    nc.sync.dma_start(wo_sb[:], wo_d.rearrange("(oc p) c -> p oc c", p=P))
    attT = attT_pool.tile([P, OC, T], BF16)

    with (
        tc.tile_pool(name="exp_pool", bufs=3) as exp_pool,
        tc.tile_pool(name="nrm", bufs=4) as nrm_pool,
        tc.tile_pool(name="sc_ps", bufs=2, space="PSUM") as sc_ps_pool,
        tc.tile_pool(name="at_ps", bufs=2, space="PSUM") as at_ps_pool,
        tc.tile_pool(name="nrm_ps", bufs=2, space="PSUM") as nrm_ps_pool,
    ):
        # One flat software-pipelined stream over all (head, ic-block,
        # jt-group) units: AV matmuls of unit u are emitted after the
        # score matmuls of unit u+1, so the PE is never queued directly
        # behind the ACT exp it depends on.  Causal masking runs on the
        # otherwise-idle GPSIMD engine.  Block-tail normalization is
        # split: the reciprocal + numerator copy (DVE) issue right after
        # the last AV (releasing the accumulator bank early); the PE
        # broadcast matmul + final multiply are delayed TAIL_DELAY
        # flushes so the PE never waits on the ~3.4us DVE reciprocal.
        pend = None  # (ex tile, [(si, jt)], h, ic, at tile, njt)
        tails = []  # [(flushes-to-wait, (h, ic, rc, tmp))]

        def start_tail(h, ic, at):
            rc = nrm_pool.tile([1, 512], BF16, tag="rc")
            with nc.allow_low_precision(reason="softmax recip bcast"):
                nc.vector.reciprocal(rc[:], at[64:65, :])
            tmp = nrm_pool.tile([64, 512], F32, tag="tmp")
            nc.vector.tensor_copy(tmp[:], at[0:64, :])
            return (h, ic, rc, tmp)

        def emit_tail(h, ic, rc, tmp):
            base = (h % 2) * 64
            oc = h // 2
            # broadcast recip row into a psum tile (K=1 outer product)
            rcb = nrm_ps_pool.tile([64, 512], F32)
            nc.tensor.matmul(rcb[:], ones1[:, 0:64], rc[:], start=True, stop=True)
            nc.vector.tensor_mul(
                attT[base : base + D, oc, ic * 512 : (ic + 1) * 512],
                tmp[:],
                rcb[:],
            )

        def flush_pend():
            nonlocal pend, tails
            if pend is None:
                return
            ex, grp, h, ic, at, njt = pend
            pend = None
            due = [t for t in tails if t[0] <= 0]
            tails = [(k - 1, args) for k, args in tails if k > 0]
            for _, args in due:
                emit_tail(*args)
            for si, jt in grp:
                nc.tensor.matmul(
                    at[:],
                    vxp[:, jt, h, :],
                    ex[:, si, :],
                    start=(jt == 0),
                    stop=(jt == njt - 1),
                )
            if grp[-1][1] == njt - 1:
                tails.append((TAIL_DELAY, start_tail(h, ic, at)))

        for oc in range(OC):
            for hh in range(2):
                h = oc * 2 + hh
                base = hh * 64
                for ic in range(T4):
                    njt = ic * 4 + 4
                    at = at_ps_pool.tile([P, 512], F32)
                    for g0 in range(0, njt, GROUP):
                        grp = list(enumerate(range(g0, min(g0 + GROUP, njt))))
                        n = len(grp)
                        sc = sc_ps_pool.tile([P, GROUP, 512], F32)
                        for si, jt in grp:
                            nc.tensor.matmul(
                                sc[:, si, :],
                                kTp[:, h, jt * P : (jt + 1) * P],
                                qTp[:, h, ic * 512 : (ic + 1) * 512],
                                start=True,
                                stop=True,
                            )
                        flush_pend()
                        ex = exp_pool.tile([P, GROUP, 512], BF16)
                        nc.scalar.activation(
                            ex[:, 0:n, :], sc[:, 0:n, :], AF.Exp, scale=0.125
                        )
                        for si, jt in grp:
                            d = jt - ic * 4
                            if d >= 0:
                                w = (d + 1) * P
                                nc.gpsimd.tensor_mul(
                                    ex[:, si, 0:w],
                                    ex[:, si, 0:w],
                                    stair_sb[:, 512 - d * P : 512 - d * P + w],
                                )
                        pend = (ex, grp, h, ic, at, njt)
        flush_pend()
        for _, args in sorted(tails, key=lambda t: t[0]):
            emit_tail(*args)

    # ---------------- out-projection ----------------
    with (
        tc.tile_pool(name="op_ps", bufs=4, space="PSUM") as op_ps,
        tc.tile_pool(name="ob_pool", bufs=4) as ob_pool,
    ):
        for s16 in range(NT):
            for ch in range(2):
                ps = op_ps.tile([P, 512], F32)
                for oc in range(OC):
                    nc.tensor.matmul(
                        ps[:],
                        attT[:, oc, s16 * P : (s16 + 1) * P],
                        wo_sb[:, oc, ch * 512 : (ch + 1) * 512],
                        start=(oc == 0),
                        stop=(oc == OC - 1),
                    )
                ob = ob_pool.tile([P, 512], F32)
                nc.vector.tensor_copy(ob[:], ps[:])
                eng[(s16 + ch) % 2].dma_start(
                    outp[s16 * P : (s16 + 1) * P, ch * 512 : (ch + 1) * 512],
                    ob[:],
                )


def build(T=2048):
    nc = bass.Bass()
    with tile.TileContext(nc) as tc:
        with ExitStack() as ctx:
            _emit(nc, tc, ctx, T)
    return nc


def make_stair():
    j = np.arange(P)[:, None]
    u = np.arange(1024)[None, :]
    return (u >= j + 512).astype(ml_dtypes.bfloat16)


def make_in_maps(x, wq, bq, wk, bk, wv, bv, wo):
    bf = ml_dtypes.bfloat16
    stair = make_stair()
    in_maps = []
    for c in range(8):
        b, g = c // 2, c % 2
        sl = slice(g * CL, (g + 1) * CL)
        in_maps.append(
            {
                "xb": np.ascontiguousarray(x[b]).astype(bf),
                "wq": np.ascontiguousarray(wq[:, sl]).astype(bf),
                "wk": np.ascontiguousarray(wk[:, sl]).astype(bf),
                "wv": np.ascontiguousarray(wv[:, sl]).astype(bf),
                "wo": np.ascontiguousarray(wo[sl, :]).astype(bf),
                "bq": np.ascontiguousarray(bq[sl]),
                "bk": np.ascontiguousarray(bk[sl]),
                "bv": np.ascontiguousarray(bv[sl]).astype(bf),
                "stair": stair,
            }
        )
    return in_maps


_cache = {}


def _split_multi_waits(bir_json: bytes) -> bytes:
    """Split instructions carrying >1 sync waits into single-wait NoOp
    chains on the same engine queue.  The TPB instruction encoding has one
    wait slot; this walrus build refuses multi-wait instructions instead
    of splitting them itself."""
    import orjson

    m = orjson.loads(bir_json)
    n = 0
    for fn in m.get("functions", []):
        for blk in fn.get("blocks", []):
            out = []
            for inst in blk.get("instructions", []):
                si = inst.get("sync_info")
                waits = si.get("on_wait") if si else None
                if waits and len(waits) > 1:
                    for w in waits[:-1]:
                        n += 1
                        out.append(
                            {
                                "debug": inst.get("debug", {}),
                                "engine": inst["engine"],
                                "ins": [],
                                "outs": [],
                                "name": f"{inst['name']}_sw{n}",
                                "opcode": "NoOp",
                                "text_hint": "split_wait",
                                "sync_info": {"on_wait": [w], "on_update": []},
                            }
                        )
                    si["on_wait"] = [waits[-1]]
                out.append(inst)
            blk["instructions"] = out
    return orjson.dumps(m)


def _install_compile_patch():
    import concourse.bass_utils as bu

    if getattr(bu, "_split_waits_patched", False):
        return
    orig = bu.compile_bir_kernel

    def patched(bir_json, tmpdir, neff_name="file.neff"):
        return orig(_split_multi_waits(bir_json), tmpdir, neff_name)

    bu.compile_bir_kernel = patched
    bu._split_waits_patched = True
    try:
        import concourse.bass2jax as b2j

        b2j.compile_bir_kernel = patched
    except ImportError:
        pass


def kernel(x, wq, bq, wk, bk, wv, bv, wo, bo):
    from concourse.bass_utils import run_bass_kernel_spmd

    _install_compile_patch()

    x = np.asarray(x, np.float32)
    args = [np.asarray(a, np.float32) for a in (wq, bq, wk, bk, wv, bv, wo, bo)]
    wq, bq, wk, bk, wv, bv, wo, bo = args
    B, T, _ = x.shape

    if "nc" not in _cache:
        _cache["nc"] = build(T)
    nc = _cache["nc"]

    in_maps = make_in_maps(x, wq, bq, wk, bk, wv, bv, wo)
    res = run_bass_kernel_spmd(nc, in_maps, core_ids=list(range(8)))
    out = np.empty((B, T, C), np.float32)
    for b in range(B):
        out[b] = res.results[2 * b]["outp"] + res.results[2 * b + 1]["outp"] + bo
    return out


# revision 19
# speedup vs baseline: 1.0652x; 1.0652x over previous
"""Causal self-attention Trainium2 kernel.

Sharding: 8 cores = (4 batches) x (2 head-groups of 8 heads).
Each core: projections for its 512 channels, causal attention for its 8
heads over its batch, partial out-projection over its 512 channels.
Host: sums the two partials per batch and adds the output bias.

All matmul operands are bf16 (accumulation in fp32 PSUM); softmax,
normalization and the output partials stay fp32.

v2 structure (PE-density focused):
  - input DMAs split per 128-channel chunk and interleaved so the first
    projection matmul issues within a few us
  - attention is one flat software-pipelined stream: score-matmul unit
    u+1 issues before AV-matmul unit u, so the PE never waits on the
    ACT exp of the unit it just produced (keeps HAM clock at 2.4 GHz)
  - softmax normalization: reciprocal_approx_fast (DVE) + SBUF
    broadcast DMA + one DVE multiply; no PE broadcast matmul, no
    numerator copy

Layouts on core (b = fixed batch, channels o in [0,512) local):
  xT   [128f, 8fc, 2048t] bf16 - DMA transpose from DRAM, per fc chunk
  qT/kT [128o, 4oc, 2048t] bf16 - head h = oc*2+hh on partitions hh*64..+64
  vx   [128t, 16tj, 8h*65] bf16 - v natural + ones column per head
  scores^T tiles [128j, 512i] f32 psum -> exp on ACT (scale=1/8) -> bf16
  at accum psum [65, 512i] f32: rows 0..63 head out, row 64 denom
  out  psum [128t, 512c] f32 -> sbuf -> DRAM partial
"""

from contextlib import ExitStack

import ml_dtypes
import numpy as np

import concourse.bass as bass
import concourse.mybir as mybir
import concourse.tile as tile

P = 128
C = 1024  # d_model
CL = 512  # local channels (8 heads * 64)
D = 64  # head dim
NH = 8  # local heads
FC = C // P  # 8 f-chunks
OC = CL // P  # 4 o-chunks
F32 = mybir.dt.float32
BF16 = mybir.dt.bfloat16
AF = mybir.ActivationFunctionType
GROUP = 2  # score jt-tiles per exp call (2 psum banks, double buffered)
TAIL_DELAY = 7  # flushes between a block's last AV and its norm broadcast


def _emit(nc, tc, ctx, T):
    NT = T // P  # 128-token chunks
    T4 = T // 512  # 512-token chunks

    xb = nc.dram_tensor("xb", [T, C], BF16, kind="ExternalInput")
    wq_d = nc.dram_tensor("wq", [C, CL], BF16, kind="ExternalInput")
    wk_d = nc.dram_tensor("wk", [C, CL], BF16, kind="ExternalInput")
    wv_d = nc.dram_tensor("wv", [C, CL], BF16, kind="ExternalInput")
    wo_d = nc.dram_tensor("wo", [CL, C], BF16, kind="ExternalInput")
    bq_d = nc.dram_tensor("bq", [CL], F32, kind="ExternalInput")
    bk_d = nc.dram_tensor("bk", [CL], F32, kind="ExternalInput")
    bv_d = nc.dram_tensor("bv", [CL], BF16, kind="ExternalInput")
    stair_d = nc.dram_tensor("stair", [P, 1024], BF16, kind="ExternalInput")
    outp = nc.dram_tensor("outp", [T, C], F32, kind="ExternalOutput")

    const = ctx.enter_context(tc.tile_pool(name="const", bufs=1))
    ones1 = const.tile([1, P], BF16)
    nc.gpsimd.memset(ones1[:], 1.0)

    # q/k per head padded to 128 partitions (rows 64:128 zero) and the AV
    # stationary padded to 128 columns: the PE activity monitor only sees
    # full-row/column matmuls as "busy", and half-utilized matmuls leave
    # the clock gate at 1.2 GHz.  The zero padding costs no matmul time
    # (cost is column count) but keeps the array at 2.4 GHz.
    qkv = ctx.enter_context(tc.tile_pool(name="qkv", bufs=1))
    qTp = qkv.tile([P, NH, T], BF16)
    kTp = qkv.tile([P, NH, T], BF16)
    vxp = qkv.tile([P, NT, NH, P], BF16)
    nc.gpsimd.memset(qTp[D:P, :, :], 0.0)
    nc.gpsimd.memset(kTp[D:P, :, :], 0.0)
    nc.gpsimd.memset(vxp[:, :, :, 64:65], 1.0)
    nc.gpsimd.memset(vxp[:, :, :, 65:P], 0.0)

    # ---------------- projections ----------------
    with (
        tc.tile_pool(name="wpool", bufs=1) as wpool,
        tc.tile_pool(name="xT_pool", bufs=1) as xT_pool,
        tc.tile_pool(name="pj_ps", bufs=8, space="PSUM") as pj_ps,
    ):
        xT = xT_pool.tile([P, FC, T], BF16)
        wq_sb = wpool.tile([P, FC, CL], BF16)
        wk_sb = wpool.tile([P, FC, CL], BF16)
        wv_sb = wpool.tile([P, FC, CL], BF16)
        xbr = xb.rearrange("t (fc p) -> t fc p", p=P)
        wqr = wq_d.rearrange("(fc p) o -> fc p o", p=P)
        wkr = wk_d.rearrange("(fc p) o -> fc p o", p=P)
        wvr = wv_d.rearrange("(fc p) o -> fc p o", p=P)
        # interleave so chunk fc of x/wq/wk arrives before chunk fc+1,
        # alternating between the two hwdge rings (SP and ACT) so neither
        # ring's serial issue rate starves the PE.
        eng = [nc.sync, nc.scalar]
        for fc in range(FC):
            nc.sync.dma_start(
                xT[:, fc, 0 : T // 2], xbr[0 : T // 2, fc, :], transpose=True
            )
            nc.sync.dma_start(
                xT[:, fc, T // 2 : T], xbr[T // 2 : T, fc, :], transpose=True
            )
            nc.scalar.dma_start(wq_sb[:, fc, :], wqr[fc])
            nc.scalar.dma_start(wk_sb[:, fc, :], wkr[fc])
        bq_sb = const.tile([P, OC], F32)
        nc.scalar.dma_start(bq_sb[:], bq_d.rearrange("(oc p) -> p oc", p=P))
        bk_sb = const.tile([P, OC], F32)
        nc.scalar.dma_start(bk_sb[:], bk_d.rearrange("(oc p) -> p oc", p=P))
        for fc in range(FC):
            eng[(fc + 1) % 2].dma_start(wv_sb[:, fc, :], wvr[fc])
        bv_sb = const.tile([1, CL], BF16)
        nc.scalar.dma_start(bv_sb[:], bv_d.rearrange("(a c) -> a c", a=1))
        stair_sb = const.tile([P, 1024], BF16)
        nc.scalar.dma_start(stair_sb[:], stair_d[:])

        for oc in range(OC):
            # q and k interleaved per fc chunk so the PE consumes each
            # arriving x/w chunk at twice the rate (8 matmuls per chunk)
            pss = {
                w: [
                    pj_ps.tile([P, 512], F32, tag="pj", name=f"pj{oc}{w}{tt}")
                    for tt in range(T4)
                ]
                for w in "qk"
            }
            for fc in range(FC):
                for w, w_sb in (("q", wq_sb), ("k", wk_sb)):
                    for tt in range(T4):
                        nc.tensor.matmul(
                            pss[w][tt][:],
                            w_sb[:, fc, oc * P : (oc + 1) * P],
                            xT[:, fc, tt * 512 : (tt + 1) * 512],
                            start=(fc == 0),
                            stop=(fc == FC - 1),
                        )
            for w, b_sb, dT in (("q", bq_sb, qTp), ("k", bk_sb, kTp)):
                for tt in range(T4):
                    for hh in range(2):
                        nc.vector.tensor_scalar_add(
                            dT[0:D, oc * 2 + hh, tt * 512 : (tt + 1) * 512],
                            pss[w][tt][hh * D : (hh + 1) * D, :],
                            b_sb[hh * D : (hh + 1) * D, oc : oc + 1],
                        )
        for s in range(NT):
            ps = pj_ps.tile([P, 512], F32, tag="pj", name=f"pjv{s}")
            for fc in range(FC):
                nc.tensor.matmul(
                    ps[:],
                    xT[:, fc, s * P : (s + 1) * P],
                    wv_sb[:, fc, :],
                    start=(fc == 0),
                    stop=False,
                )
            nc.tensor.matmul(
                ps[:],
                ones1[:],
                bv_sb[:],
                start=False,
                stop=True,
            )
            nc.vector.tensor_copy(
                vxp[:, s, :, 0:64],
                ps[:].rearrange("p (h d) -> p h d", d=D),
            )

    # ---------------- attention ----------------
    wo_pool = ctx.enter_context(tc.tile_pool(name="wo_pool", bufs=1))
    attT_pool = ctx.enter_context(tc.tile_pool(name="attT_pool", bufs=1))
    wo_sb = wo_pool.tile([P, OC, C], BF16)
    nc.sync.dma_start(wo_sb[:], wo_d.rearrange("(oc p) c -> p oc c", p=P))
    attT = attT_pool.tile([P, OC, T], BF16)

    with (
        tc.tile_pool(name="exp_pool", bufs=3) as exp_pool,
        tc.tile_pool(name="nrm", bufs=4) as nrm_pool,
        tc.tile_pool(name="sc_ps", bufs=2, space="PSUM") as sc_ps_pool,
        tc.tile_pool(name="at_ps", bufs=2, space="PSUM") as at_ps_pool,
        tc.tile_pool(name="nrm_ps", bufs=2, space="PSUM") as nrm_ps_pool,
    ):
        # One flat software-pipelined stream over all (head, ic-block,
        # jt-group) units.  AV matmuls of unit u are emitted two units
        # later, covering the scores -> ACT exp -> GPSIMD band-mask
        # dependency chain so the PE never queues behind it.  Diagonal
        # tiles are processed at partial width [128d:512] (scores, AV)
        # with only the 128-wide triangular band masked; exp runs full
        # width but the unmasked garbage is never read by the partial AV.
        # Block-tail normalization is staged: reciprocal + numerator
        # copy (DVE) issue right after the last AV (freeing the psum
        # accumulator); the PE broadcast waits BC_DELAY flushes and the
        # final multiply MUL_DELAY flushes, decoupling the DVE queue
        # head from future PE instructions.
        pendq = []  # [(ex tile, [(si, jt)], h, ic, at tile, njt)]
        bcasts = []  # [(flushes-to-wait, (h, ic, rc, tmp, [rcb]))]
        muls = []
        BC_DELAY, MUL_DELAY = 4, 8

        def off_of(jt, ic):
            return max(0, jt - ic * 4) * P

        def emit_av(ex, grp, h, ic, at, njt):
            for si, jt in grp:
                off = off_of(jt, ic)
                nc.tensor.matmul(
                    at[:, off:512],
                    vxp[:, jt, h, :],
                    ex[:, si, off:512],
                    start=(jt == 0),
                    stop=(jt == njt - 1),
                )

        def start_tail(h, ic, at):
            rc = nrm_pool.tile([1, 512], BF16, tag="rc")
            with nc.allow_low_precision(reason="softmax recip bcast"):
                nc.vector.reciprocal(rc[:], at[64:65, :])
            tmp = nrm_pool.tile([64, 512], F32, tag="tmp")
            nc.vector.tensor_copy(tmp[:], at[0:64, :])
            return [h, ic, rc, tmp]

        def emit_bcast(args):
            rcb = nrm_ps_pool.tile([64, 512], F32)
            nc.tensor.matmul(rcb[:], ones1[:, 0:64], args[2][:], start=True, stop=True)
            args.append(rcb)

        def emit_mul(args):
            h, ic, rc, tmp, rcb = args
            base = (h % 2) * 64
            nc.vector.tensor_mul(
                attT[base : base + D, h // 2, ic * 512 : (ic + 1) * 512],
                tmp[:],
                rcb[:],
            )

        def flush_pend(force=False):
            nonlocal pendq, bcasts, muls
            if not pendq or (len(pendq) <= 2 and not force):
                return
            for lst, emitter in ((muls, emit_mul), (bcasts, emit_bcast)):
                for k, args in lst:
                    if k <= 0:
                        emitter(args)
                lst[:] = [(k - 1, args) for k, args in lst if k > 0]
            ex, grp, h, ic, at, njt = pendq.pop(0)
            emit_av(ex, grp, h, ic, at, njt)
            if grp[-1][1] == njt - 1:
                args = start_tail(h, ic, at)
                bcasts.append((BC_DELAY, args))
                muls.append((MUL_DELAY, args))

        for oc in range(OC):
            for hh in range(2):
                h = oc * 2 + hh
                for ic in range(T4):
                    njt = ic * 4 + 4
                    at = at_ps_pool.tile([P, 512], F32)
                    for g0 in range(0, njt, GROUP):
                        grp = list(enumerate(range(g0, min(g0 + GROUP, njt))))
                        n = len(grp)
                        sc = sc_ps_pool.tile([P, GROUP, 512], F32)
                        for si, jt in grp:
                            off = off_of(jt, ic)
                            nc.tensor.matmul(
                                sc[:, si, off:512],
                                kTp[:, h, jt * P : (jt + 1) * P],
                                qTp[:, h, ic * 512 + off : (ic + 1) * 512],
                                start=True,
                                stop=True,
                            )
                        ex = exp_pool.tile([P, GROUP, 512], BF16)
                        nc.scalar.activation(
                            ex[:, 0:n, :], sc[:, 0:n, :], AF.Exp, scale=0.125
                        )
                        for si, jt in grp:
                            d = jt - ic * 4
                            if d >= 0:
                                off = d * P
                                nc.gpsimd.tensor_mul(
                                    ex[:, si, off : off + P],
                                    ex[:, si, off : off + P],
                                    stair_sb[:, 512:640],
                                )
                        pendq.append((ex, grp, h, ic, at, njt))
                        flush_pend()
        while pendq or bcasts or muls:
            if pendq:
                flush_pend(force=True)
            else:
                for lst, emitter in ((muls, emit_mul), (bcasts, emit_bcast)):
                    for k, args in lst:
                        if k <= 0:
                            emitter(args)
                    lst[:] = [(k - 1, args) for k, args in lst if k > 0]

    # ---------------- out-projection ----------------
    with (
        tc.tile_pool(name="op_ps", bufs=4, space="PSUM") as op_ps,
        tc.tile_pool(name="ob_pool", bufs=4) as ob_pool,
    ):
        for s16 in range(NT):
            for ch in range(2):
                ps = op_ps.tile([P, 512], F32)
                for oc in range(OC):
                    nc.tensor.matmul(
                        ps[:],
                        attT[:, oc, s16 * P : (s16 + 1) * P],
                        wo_sb[:, oc, ch * 512 : (ch + 1) * 512],
                        start=(oc == 0),
                        stop=(oc == OC - 1),
                    )
                ob = ob_pool.tile([P, 512], F32)
                nc.vector.tensor_copy(ob[:], ps[:])
                eng[(s16 + ch) % 2].dma_start(
                    outp[s16 * P : (s16 + 1) * P, ch * 512 : (ch + 1) * 512],
                    ob[:],
                )


def build(T=2048):
    nc = bass.Bass()
    with tile.TileContext(nc) as tc:
        with ExitStack() as ctx:
            _emit(nc, tc, ctx, T)
    return nc


def make_stair():
    j = np.arange(P)[:, None]
    u = np.arange(1024)[None, :]
    return (u >= j + 512).astype(ml_dtypes.bfloat16)


def make_in_maps(x, wq, bq, wk, bk, wv, bv, wo):
    bf = ml_dtypes.bfloat16
    stair = make_stair()
    in_maps = []
    for c in range(8):
        b, g = c // 2, c % 2
        sl = slice(g * CL, (g + 1) * CL)
        in_maps.append(
            {
                "xb": np.ascontiguousarray(x[b]).astype(bf),
                "wq": np.ascontiguousarray(wq[:, sl]).astype(bf),
                "wk": np.ascontiguousarray(wk[:, sl]).astype(bf),
                "wv": np.ascontiguousarray(wv[:, sl]).astype(bf),
                "wo": np.ascontiguousarray(wo[sl, :]).astype(bf),
                "bq": np.ascontiguousarray(bq[sl]),
                "bk": np.ascontiguousarray(bk[sl]),
                "bv": np.ascontiguousarray(bv[sl]).astype(bf),
                "stair": stair,
            }
        )
    return in_maps


_cache = {}


def _split_multi_waits(bir_json: bytes) -> bytes:
    """Split instructions carrying >1 sync waits into single-wait NoOp
    chains on the same engine queue.  The TPB instruction encoding has one
    wait slot; this walrus build refuses multi-wait instructions instead
    of splitting them itself."""
    import orjson

    m = orjson.loads(bir_json)
    n = 0
    for fn in m.get("functions", []):
        for blk in fn.get("blocks", []):
            out = []
            for inst in blk.get("instructions", []):
                si = inst.get("sync_info")
                waits = si.get("on_wait") if si else None
                if waits and len(waits) > 1:
                    for w in waits[:-1]:
                        n += 1
                        out.append(
                            {
                                "debug": inst.get("debug", {}),
                                "engine": inst["engine"],
                                "ins": [],
                                "outs": [],
                                "name": f"{inst['name']}_sw{n}",
                                "opcode": "NoOp",
                                "text_hint": "split_wait",
                                "sync_info": {"on_wait": [w], "on_update": []},
                            }
                        )
                    si["on_wait"] = [waits[-1]]
                out.append(inst)
            blk["instructions"] = out
    return orjson.dumps(m)


def _install_compile_patch():
    import concourse.bass_utils as bu

    if getattr(bu, "_split_waits_patched", False):
        return
    orig = bu.compile_bir_kernel

    def patched(bir_json, tmpdir, neff_name="file.neff"):
        return orig(_split_multi_waits(bir_json), tmpdir, neff_name)

    bu.compile_bir_kernel = patched
    bu._split_waits_patched = True
    try:
        import concourse.bass2jax as b2j

        b2j.compile_bir_kernel = patched
    except ImportError:
        pass


def kernel(x, wq, bq, wk, bk, wv, bv, wo, bo):
    from concourse.bass_utils import run_bass_kernel_spmd

    _install_compile_patch()

    x = np.asarray(x, np.float32)
    args = [np.asarray(a, np.float32) for a in (wq, bq, wk, bk, wv, bv, wo, bo)]
    wq, bq, wk, bk, wv, bv, wo, bo = args
    B, T, _ = x.shape

    if "nc" not in _cache:
        _cache["nc"] = build(T)
    nc = _cache["nc"]

    in_maps = make_in_maps(x, wq, bq, wk, bk, wv, bv, wo)
    res = run_bass_kernel_spmd(nc, in_maps, core_ids=list(range(8)))
    out = np.empty((B, T, C), np.float32)
    for b in range(B):
        out[b] = res.results[2 * b]["outp"] + res.results[2 * b + 1]["outp"] + bo
    return out
